# revision 24
# baseline (speedup 1.0000x reference)
"""FM (DeepFM first+second order) multi-task kernel for Trainium2, 8 NeuronCores.

Strategy: data-parallel over batch (2048 rows/core).  The gather uses the
dedicated SWDGE dma_gather primitive (vectorized Q7 descriptor generation,
~30x cheaper per descriptor than generic indirect DMA) with int16 indices.
To fit the 100k vocab into int16 range, two vocab rows are packed per 512-byte
table slot (slot id = v>>1 in [0, 50000)) and the call's base pointer sits at
the field's slot midpoint so signed indices (v>>1)-25000 in [-25000, 25000)
cover the whole field.  Each gathered slot holds both candidate rows
interleaved (elem[2e+g] = emb2[2s+g, e], fp16) plus both emb1 scalars; a
host-precomputed one-hot (v&1) mask selects the right candidate on-chip:
DVE mask-multiply + unit-stride pair reduce, ACT squares, tiny finalization.
26 dma_gather calls per core (one per field, 2176 idxs each incl. pad chunk
that keeps the trailing index non-negative so the ucode's trailing-trim
doesn't drop real rows).
"""

import os
import sys

import numpy as np

if "/opt/trn_rl_repo" not in sys.path:
    sys.path.insert(0, "/opt/trn_rl_repo")

N_DENSE = 13
F = 26           # n sparse fields
V = 100000       # vocab
E = 64           # emb dim
B = 16384        # global batch
N_CORES = 8
BC = B // N_CORES        # 2048 batch rows per core
TB = 128                 # SBUF partitions
NC_ = BC // TB           # 16 sample chunks per core (out free dim)
G = 2                    # vocab rows packed per slot
S_FIELD = V // G         # 50000 slots per field
MID = S_FIELD // 2       # 25000: signed-index midpoint
ROWE = 256               # fp16 elems per slot (512 B)
NIDX = BC + 16           # 2064: 2048 real + one 16-wide pad column
NCOL = (NIDX + 15) // 16  # 129 wrapped idx columns
NCHUNK = (NIDX + TB - 1) // TB  # 17 (out tile rounds up to 128)
AUXW = 19                # aux scalar vector width

_NC_CACHE = {}


def build_nc(debug=False):
    import concourse.bass as bass
    import concourse.tile as tile
    from concourse import bacc, mybir
    from contextlib import ExitStack

    f32 = mybir.dt.float32
    f16 = mybir.dt.float16
    i16 = mybir.dt.int16
    Square = mybir.ActivationFunctionType.Square
    Sigmoid = mybir.ActivationFunctionType.Sigmoid
    add = mybir.AluOpType.add

    nc = bacc.Bacc(
        "TRN2",
        target_bir_lowering=False,
        debug=debug,
        num_devices=N_CORES,
        num_swdge_queues=4,
        dynamic_dma_scratch_size=65536,
    )

    table = nc.dram_tensor("table", [F * S_FIELD, ROWE], f16, kind="ExternalInput").ap()
    idxs = nc.dram_tensor("idxs", [TB, F, NCOL], i16, kind="ExternalInput").ap()
    masks = nc.dram_tensor("masks", [TB, F, NC_, G], f16, kind="ExternalInput").ap()
    dense = nc.dram_tensor("dense", [TB, NC_, N_DENSE], f32, kind="ExternalInput").ap()
    aux = nc.dram_tensor("aux", [TB, AUXW], f32, kind="ExternalInput").ap()
    fin = nc.dram_tensor("finish", [TB, NC_], f32, kind="ExternalOutput").ap()
    lik = nc.dram_tensor("like", [TB, NC_], f32, kind="ExternalOutput").ap()

    with tile.TileContext(nc) as tc, ExitStack() as ctx:
        ctx.enter_context(
            nc.allow_low_precision(
                reason="fp16 pair-select/field sums; 2e-2 output tolerance"
            )
        )
        singles = ctx.enter_context(tc.tile_pool(name="singles", bufs=1))
        gpool = ctx.enter_context(tc.tile_pool(name="g", bufs=6))
        wpool = ctx.enter_context(tc.tile_pool(name="w", bufs=3))
        opool = ctx.enter_context(tc.tile_pool(name="o", bufs=2))

        # idx load first: the first dma_gather only waits on this transfer
        idx_t = singles.tile([TB, F, NCOL], i16)
        nc.sync.dma_start(out=idx_t[:], in_=idxs[:])
        aux_t = singles.tile([TB, AUXW], f32)
        nc.sync.dma_start(out=aux_t[:], in_=aux[:])
        mask_t = singles.tile([TB, F, NC_, G], f16)
        nc.sync.dma_start(out=mask_t[:], in_=masks[:])
        d_t = singles.tile([TB, NC_, N_DENSE], f32)
        nc.sync.dma_start(out=d_t[:], in_=dense[:])

        # accumulators over fields; S has E+1 cols: col E = emb1 sum (the slot
        # layout places the emb1 pair right after the 64 interleaved emb2
        # pairs, so one fused mask-select covers both)
        S_t = singles.tile([TB, NC_, E + 1], f16)
        Q_t = singles.tile([TB, NC_, E], f16)     # sum of squared selected rows
        nc.vector.memset(S_t[:], 0.0)
        nc.vector.memset(Q_t[:], 0.0)

        # dense linear head is independent of the gathers: compute it up front
        # so the post-gather tail only runs the logit combine
        wb = aux_t[:, 0:N_DENSE].unsqueeze(1).broadcast_to([TB, NC_, N_DENSE])
        dsc = singles.tile([TB, NC_, N_DENSE], f32)
        nc.vector.tensor_mul(dsc[:], d_t[:], wb)
        do = singles.tile([TB, NC_], f32)
        nc.vector.tensor_reduce(
            out=do[:], in_=dsc[:], axis=mybir.AxisListType.X, op=add
        )

        for f in range(F):
            g_t = gpool.tile([TB, NCHUNK, ROWE], f16)
            nc.gpsimd.dma_gather(
                g_t[:],
                table[f * S_FIELD + MID :, :],
                idx_t[:, f, :],
                NIDX,
                NIDX,
                ROWE,
                single_packet=False,
                queue_num=f % 4,
            )
            # fused mask-select over the 65 (e, pair) positions: cols 0:64 are
            # emb2, col 64 is emb1 (slot elems [0:130) = 65 interleaved pairs)
            mb = mask_t[:, f].unsqueeze(2).broadcast_to([TB, NC_, E + 1, G])
            m1 = wpool.tile([TB, NC_, E + 1, G], f16)
            g2 = g_t[:, 0:NC_, 0 : (E + 1) * G].rearrange(
                "p c (e g) -> p c e g", g=G
            )
            nc.vector.tensor_mul(m1[:], g2, mb)
            sel = wpool.tile([TB, NC_, E + 1], f16)
            nc.vector.tensor_reduce(
                out=sel[:], in_=m1[:], axis=mybir.AxisListType.X, op=add
            )
            nc.vector.tensor_add(S_t[:], S_t[:], sel[:])
            sq = wpool.tile([TB, NC_, E], f16)
            nc.scalar.activation(
                out=sq[:], in_=sel[:, :, 0:E], func=Square, scale=1.0,
            )
            nc.vector.tensor_add(Q_t[:], Q_t[:], sq[:])

        # ---- finalize (per sample = (partition, chunk)) ----
        # ss = sum_e S^2 (square on ACT, overlapping qs reduce on DVE)
        s2 = wpool.tile([TB, NC_, E], f32)
        nc.scalar.activation(
            out=s2[:], in_=S_t[:, :, 0:E], func=Square, scale=1.0,
        )
        qs = opool.tile([TB, NC_], f32)
        nc.vector.tensor_reduce(
            out=qs[:], in_=Q_t[:], axis=mybir.AxisListType.X, op=add
        )
        ss = opool.tile([TB, NC_], f32)
        nc.vector.tensor_reduce(out=ss[:], in_=s2[:], axis=mybir.AxisListType.X, op=add)
        # logits = do + b_dense + S1 + 0.5*(ss - qs)
        df = opool.tile([TB, NC_], f32)
        nc.vector.tensor_sub(df[:], ss[:], qs[:])
        l1 = opool.tile([TB, NC_], f32)
        nc.vector.tensor_scalar_mul(l1[:], df[:], 0.5)
        l2 = opool.tile([TB, NC_], f32)
        nc.vector.tensor_add(l2[:], l1[:], do[:])
        l3 = opool.tile([TB, NC_], f32)
        nc.vector.tensor_add(l3[:], l2[:], S_t[:, :, E])
        bb = aux_t[:, 13:14].broadcast_to([TB, NC_])
        lg = opool.tile([TB, NC_], f32)
        nc.vector.tensor_add(lg[:], l3[:], bb)

        fin_t = opool.tile([TB, NC_], f32)
        lik_t = opool.tile([TB, NC_], f32)
        nc.scalar.activation(
            out=fin_t[:], in_=lg[:], func=Sigmoid,
            scale=aux_t[:, 14:15], bias=aux_t[:, 15:16],
        )
        nc.scalar.activation(
            out=lik_t[:], in_=lg[:], func=Sigmoid,
            scale=aux_t[:, 16:17], bias=aux_t[:, 17:18],
        )
        nc.sync.dma_start(out=fin[:], in_=fin_t[:])
        nc.sync.dma_start(out=lik[:], in_=lik_t[:])

    nc.compile()
    return nc


def _get_nc():
    if "nc" not in _NC_CACHE:
        _NC_CACHE["nc"] = build_nc(debug=False)
    return _NC_CACHE["nc"]


def _prepare_inputs(sparse_inputs, dense_inputs, emb1, emb2, W_dense, b_dense,
                    W_finish, b_finish, W_like, b_like):
    sparse_inputs = np.asarray(sparse_inputs)
    dense_inputs = np.asarray(dense_inputs, dtype=np.float32)
    emb1 = np.asarray(emb1, dtype=np.float32)
    emb2 = np.asarray(emb2, dtype=np.float32)

    # slot table: [F*S_FIELD, 256] fp16; slot s of field f covers v in
    # {2s, 2s+1}: elems[2e+g] = emb2[f, 2s+g, e]; elems[128+g] = emb1[f, 2s+g]
    T = np.zeros((F * S_FIELD, ROWE), dtype=np.float16)
    e2 = emb2.astype(np.float16).reshape(F, S_FIELD, G, E).transpose(0, 1, 3, 2)
    T[:, 0 : E * G] = e2.reshape(F * S_FIELD, E * G)
    T[:, E * G : E * G + G] = (
        emb1.astype(np.float16).reshape(F, S_FIELD, G).reshape(F * S_FIELD, G)
    )

    aux = np.zeros((TB, AUXW), dtype=np.float32)
    aux[:, 0:N_DENSE] = np.asarray(W_dense, dtype=np.float32).reshape(-1)
    aux[:, 13] = np.float32(np.asarray(b_dense).reshape(-1)[0])
    aux[:, 14] = np.float32(np.asarray(W_finish).reshape(-1)[0])
    aux[:, 15] = np.float32(np.asarray(b_finish).reshape(-1)[0])
    aux[:, 16] = np.float32(np.asarray(W_like).reshape(-1)[0])
    aux[:, 17] = np.float32(np.asarray(b_like).reshape(-1)[0])

    v_all = sparse_inputs.astype(np.int64)          # [B, F] in [0, V)
    slot_all = (v_all >> 1) - MID                   # signed slot offsets
    gsel_all = (v_all & 1).astype(np.int64)         # which row of the pair

    in_maps = []
    j = np.arange(BC)
    ch16, col = (j % 16), (j // 16)
    for c in range(N_CORES):
        sl = slice(c * BC, (c + 1) * BC)
        slot = slot_all[sl]                         # [BC, F]
        gsel = gsel_all[sl]
        # idx16 [128, F, NIDX//16]: position j -> (partition j%16 (+16k), col j//16)
        # pad: position 2048 holds a positive sentinel (caps the ucode's
        # trailing-negative trim so real negative indices are never trimmed);
        # the remaining pad is -1 and gets trimmed -> no SDMA traffic for it
        tmp = np.full((16, F, NCOL), -1, dtype=np.int16)
        tmp[ch16, :, col] = slot.astype(np.int16)
        tmp[0, :, BC // 16] = 0
        idx16 = np.tile(tmp, (8, 1, 1))
        # masks [128, F, NC_, G] one-hot of gsel; sample j = chunk*128 + p
        mk = np.zeros((TB, F, NC_, G), dtype=np.float16)
        p_of_j, c_of_j = (j % TB), (j // TB)
        for g in range(G):
            sel = (gsel == g)                       # [BC, F]
            mk[p_of_j[:, None], np.arange(F)[None, :], c_of_j[:, None], g] = (
                sel.astype(np.float16)
            )
        # dense pre-transposed to [TB, NC_, ND]: sample j = chunk*128 + p
        dcore = dense_inputs[sl].reshape(NC_, TB, N_DENSE)
        in_maps.append(dict(
            table=T,
            idxs=idx16,
            masks=mk,
            dense=np.ascontiguousarray(dcore.transpose(1, 0, 2)),
            aux=aux,
        ))
    return in_maps


def _install_trace_hooks():
    """Make trace=True work in containers whose antenv stub lacks axon_hooks."""
    import sys
    import types

    try:
        from antenv.axon_hooks import get_axon_ntff_profile_hook  # noqa: F401
    except ImportError:
        mod = types.ModuleType("antenv.axon_hooks")
        mod._hook = None
        mod.set_axon_ntff_profile_hook = lambda h: setattr(mod, "_hook", h)
        mod.get_axon_ntff_profile_hook = lambda: mod._hook
        sys.modules["antenv.axon_hooks"] = mod
        import antenv

        antenv.axon_hooks = mod
        from trn_agent_boot.trn_boot import _ntff_profile_via_ctypes

        mod._hook = _ntff_profile_via_ctypes("/opt/axon/libaxon_pjrt.so")

    from concourse import bass_utils

    bass_utils.upload_artifacts = lambda tmpdir: f"local://{tmpdir}"


def run(inputs, trace=False, cores=None):
    """Run on the NeuronCores; returns ((finish, like), BassKernelResults)."""
    from concourse.bass_utils import run_bass_kernel_spmd

    if trace:
        _install_trace_hooks()
    in_maps = _prepare_inputs(**inputs)
    nc = _get_nc()
    ncores = cores if cores is not None else N_CORES
    res = run_bass_kernel_spmd(nc, in_maps[:ncores], list(range(ncores)), trace=trace)
    # device layout [TB, NC_]: sample j = chunk*128 + p lives at [p, chunk]
    fin = np.concatenate(
        [
            res.results[c]["finish"].reshape(TB, NC_).T.reshape(BC, 1)
            for c in range(ncores)
        ],
        axis=0,
    )
    lik = np.concatenate(
        [
            res.results[c]["like"].reshape(TB, NC_).T.reshape(BC, 1)
            for c in range(ncores)
        ],
        axis=0,
    )
    return (fin, lik), res


def kernel(**inputs):
    (fin, lik), _ = run(inputs, trace=bool(int(os.environ.get("KERNEL_TRACE", "0"))))
    return fin, lik
